# revision 33
# baseline (speedup 1.0000x reference)
"""Trainium2 Bass kernel for nn_MeshDeformation (GNN message passing).

Data-parallel over batch B=8 across 8 cores, one batch item per core.

Feature-major design: activations live in SBUF as xT [128 hid-part, 2 ktiles,
NP verts] so no transposes are ever needed:
  - phase M: mm = x@W vertex-major ([v,h] = xT_blk.T @ W) -> bf16 rows to a
    ping-pong HBM buffer. Software-pipelined: conv c+1's phase M is emitted
    inside conv c's gather/scatter loop (block i emitted once conv c finished
    block i, with a small lag so PE never stalls on the evacuation).
  - phase G: batched dma_gather (one SWDGE instruction per CH-tile chunk)
    pulls dst-sorted edge rows into SBUF edge-major; chunks double-buffered.
  - phase S: per 128-edge tile, 2 matmuls (hid halves): pacc_q += g_q.T @ S_t
    giving feature-major psum out [h-half, dst]; the x@L term accumulates into
    the same psum group (lhsT = L quarter, rhs = xT block); bias+ReLU fused
    into the scalar-engine evacuation (activation bias is per-partition =
    per-hid-feature here). Residual folds the *0.5 into the activation scale.
  - S matrices (val folded in) and gather indices stay resident in SBUF for
    all 10 convs; duplicate (src, dst-block) edges are merged on the host so
    each gathered row is unique per block.
  - per-conv DRAM RAW (mm writes -> gathers) is enforced by a sentinel HWDGE
    DMA (FIFO per engine) + a Pool-engine read, not an all-engine barrier.
  - final conv computes y=x@W2 into 256B half rows, gathers those, single-
    half scatter, Tanh+bias evac, PE-transpose [3,128]->[128,3], x0.1 on evac.
  - dma_gather is capped at 1024 indices per instruction on HW (64 per Q7
    core); CH=8 tiles per chunk respects that.
"""
import sys, os
sys.path.insert(0, '/opt/trn_rl_repo')
import numpy as np
import ml_dtypes

import concourse.bass as bass
import concourse.bacc as bacc
import concourse.mybir as mybir
import concourse.tile as tile
from concourse import bass_utils
from concourse.library_config import mlp as _mlp_lib

N = 6890
NP = 6912          # padded vertices (54 * 128)
NB = NP // 128     # 54 dst/vertex blocks
HID = 256
FEAT = 128
NCONV = 10         # conv1, 8 hidden convs, final conv2
CH = 8             # gather tiles per chunk (1024 rows: HW dma_gather limit)
GB = 6             # phase-M blocks per staged HBM write
MLAG = 1           # blocks of lag for interleaved next-conv phase M
RAMP = (4, 8)      # leading gather chunk sizes
DEBUG_STAGE = 0
PIPELINE = True
BIAS_AP = True

BF16 = ml_dtypes.bfloat16


def _edge_tiles(src, dst, val):
    """dst-sorted, per-dst-block 128-padded edge tiling (vectorized).

    Returns (gidx [KT*128] int16 src ids (pad=0), S [KT,128,128] f32,
    tile_block [KT] int array).
    """
    blk = (dst >> 7).astype(np.int64)
    # one gathered row per unique (block, src); duplicate edges fold into S
    pair = blk * 8192 + src                      # unique (block, src) key
    uniq, inv = np.unique(pair, return_inverse=True)
    ublk = (uniq // 8192).astype(np.int64)
    usrc = (uniq % 8192).astype(np.int64)
    counts = np.bincount(ublk, minlength=NB)     # unique rows per block
    ntiles = (counts + 127) // 128
    KT = int(ntiles.sum())
    tile_block = np.repeat(np.arange(NB), ntiles)
    block_start_row = np.concatenate([[0], np.cumsum(counts)])[:-1]
    block_start_tile = np.concatenate([[0], np.cumsum(ntiles)])[:-1]
    nrow = len(uniq)
    within = np.arange(nrow) - block_start_row[ublk]   # row slot within block
    tile_of_row = block_start_tile[ublk] + (within >> 7)
    k_of_row = within & 127
    gidx = np.zeros(KT * 128, np.int16)
    gidx[tile_of_row * 128 + k_of_row] = usrc.astype(np.int16)
    S = np.zeros((KT, 128, 128), np.float32)
    np.add.at(S, (tile_of_row[inv], k_of_row[inv], dst & 127), val)
    return gidx, S, tile_block


def _src_of(c, A, Bb):
    if c == 0:
        return Bb          # x0T in ktile-0 region
    return A if c % 2 == 1 else Bb


def _dst_of(c):
    if c == 0:
        return 'A'
    if c == 9:
        return 'final'
    return 'B' if c % 2 == 1 else 'resid'


def _build_program(tile_block, KT):
    tile_block = list(tile_block)
    chunks = []          # (jlo, nt) with a short ramp so gather-0 lands fast
    j = 0
    for nt in RAMP:
        if j < KT:
            nt = min(nt, KT - j)
            chunks.append((j, nt))
            j += nt
    while j < KT:
        nt = min(CH, KT - j)
        chunks.append((j, nt))
        j += nt
    nchunks = len(chunks)
    tile_chunk = np.zeros(KT, np.int64)
    for ci, (jlo, nt) in enumerate(chunks):
        tile_chunk[jlo:jlo + nt] = ci
    tiles_of = {}
    for j, b in enumerate(tile_block):
        tiles_of.setdefault(b, []).append(j)

    nc = bacc.Bacc("TRN2", target_bir_lowering=False, debug=False)
    bf = mybir.dt.bfloat16
    f32 = mybir.dt.float32

    x0_d = nc.dram_tensor("x0T", [128, NP], bf, kind="ExternalInput")
    wcat_d = nc.dram_tensor("wcat", [128, NCONV * 2 * HID], bf,
                            kind="ExternalInput")
    lcq_d = nc.dram_tensor("lcq", [128, NCONV * 4 * 128], bf,
                           kind="ExternalInput")
    bcol_d = nc.dram_tensor("bcol", [128, NCONV * 4], f32,
                            kind="ExternalInput")
    s_d = nc.dram_tensor("smat", [128, KT * 128], bf, kind="ExternalInput")
    gidx_d = nc.dram_tensor("gidx", [128, KT * 8], mybir.dt.int16,
                            kind="ExternalInput")
    out_d = nc.dram_tensor("out", [N, 3], f32, kind="ExternalOutput")
    if DEBUG_STAGE:
        dbg_d = nc.dram_tensor("dbg", [128, 2 * NP], bf, kind="ExternalOutput")

    from concourse.masks import make_identity

    with tile.TileContext(nc) as tc:
        with (
            tc.tile_pool(name="dram", bufs=1, space="DRAM") as dram,
            tc.tile_pool(name="res", bufs=1) as res,
            tc.tile_pool(name="gpool", bufs=4) as gpool,
            tc.tile_pool(name="stg", bufs=3) as stg,
            tc.tile_pool(name="rstg", bufs=4) as rstg,
            tc.tile_pool(name="pmp", bufs=3, space="PSUM") as pmp,
            tc.tile_pool(name="acc", bufs=3, space="PSUM") as acc,
            tc.tile_pool(name="ptp", bufs=2, space="PSUM") as ptp,
        ):
            mm_a = dram.tile([NP, HID], bf, tag="mm0")
            mm_b = dram.tile([NP, HID], bf, tag="mm1")
            mm_pp = [mm_a, mm_b]

            S_all = res.tile([128, KT * 128], bf, tag="S")
            gix = res.tile([128, KT * 8], mybir.dt.int16, tag="gix")
            A = res.tile([128, 2 * NP], bf, tag="A")
            Bb = res.tile([128, 2 * NP], bf, tag="B")
            wc = res.tile([128, NCONV * 2 * HID], bf, tag="wc")
            lcq = res.tile([128, NCONV * 4 * 128], bf, tag="lcq")
            bcol = res.tile([128, NCONV * 4], f32, tag="bcol")
            id32 = res.tile([128, 128], f32, tag="id32")
            idbf = res.tile([128, 128], bf, tag="idbf")
            ostage = res.tile([128, NB * 3], f32, tag="ostage")
            snt = res.tile([128, 8], f32, tag="snt")
            sdm = res.tile([128, 8], f32, tag="sdm")

            nc.gpsimd.load_library(_mlp_lib)
            nc.sync.dma_start(out=wc[:], in_=wcat_d[:])
            nc.sync.dma_start(out=Bb[:, 0:NP], in_=x0_d[:])
            nc.sync.dma_start(out=lcq[:], in_=lcq_d[:])
            nc.sync.dma_start(out=bcol[:], in_=bcol_d[:])
            nc.sync.dma_start(out=gix[:], in_=gidx_d[:])
            make_identity(nc, id32[:])
            nc.vector.tensor_copy(out=idbf[:], in_=id32[:])

            nregs = {}
            for (_, nt) in chunks:
                if nt not in nregs:
                    nregs[nt] = nc.gpsimd.to_reg(nt * 128)

            def xs(src_tile, h, i):
                return src_tile[:, h * NP + i * 128: h * NP + (i + 1) * 128]

            # ---- phase M emitter (per-block, staged writes of GB blocks) ----
            def m_state(c, src_tile, mm_d):
                return dict(c=c, src=src_tile, mm=mm_d, ms=None, base=0, cnt=0)

            def emit_m_block(st, i):
                c, src_tile, mm_d = st['c'], st['src'], st['mm']
                fin = 1 if c == 0 else 2
                mw = 128 if c == NCONV - 1 else HID   # final: y in cols 0:128
                pm = pmp.tile([128, HID], f32, tag="pm", name="pm")
                for h in range(fin):
                    nc.tensor.matmul(
                        out=pm[:, 0:mw], lhsT=xs(src_tile, h, i),
                        rhs=wc[:, (2 * c + h) * HID:(2 * c + h) * HID + mw],
                        start=(h == 0), stop=(h == fin - 1))
                if st['cnt'] == 0:
                    st['ms'] = stg.tile([128, GB * HID], bf, tag="ms",
                                        name="ms")
                    st['base'] = i
                sl = st['cnt']
                if i % 2 == 0:
                    nc.vector.tensor_copy(
                        out=st['ms'][:, sl * mw:(sl + 1) * mw],
                        in_=pm[:, 0:mw])
                else:
                    nc.scalar.copy(
                        out=st['ms'][:, sl * mw:(sl + 1) * mw],
                        in_=pm[:, 0:mw])
                st['cnt'] += 1
                # flush per-block near the end so the last write lands sooner
                if st['cnt'] == GB or i == NB - 1 or i >= NB - 3:
                    lo, nb = st['base'], st['cnt']
                    nc.sync.dma_start(
                        out=mm_d[lo * 128:(lo + nb) * 128, 0:mw].rearrange(
                            "(b p) h -> p b h", p=128),
                        in_=st['ms'][:, :nb * mw].rearrange(
                            "p (b h) -> p b h", h=mw))
                    st['cnt'] = 0
                if i == NB - 1:
                    nc.sync.dma_start(out=snt[:], in_=bcol_d[:, 0:8])

            # ---- gather + scatter phase for one conv ----
            def conv_gs(c, src_tile, dst_mode, mm_d, nxt):
                fin = 1 if c == 0 else 2
                final = dst_mode == 'final'
                nq = 1 if final else 2

                gtiles = {}
                issued = [0]
                fpend = []
                # final conv: y lives in cols 0:128 of the 256-wide mm rows;
                # gather only 256B per row
                gel = 128 if final else HID

                def need_chunk(k):
                    while issued[0] <= min(k + 1, nchunks - 1):
                        ci = issued[0]
                        jlo, nt = chunks[ci]
                        if c == 0:
                            # stream the resident S in step with conv0's use
                            nc.sync.dma_start(
                                out=S_all[:, jlo * 128:(jlo + nt) * 128],
                                in_=s_d[:, jlo * 128:(jlo + nt) * 128])
                        gt = gpool.tile([128, CH * HID], bf, tag="g", name="g")
                        nc.gpsimd.dma_gather(
                            gt[:, :nt * gel].rearrange("p (t e) -> p t e",
                                                       e=gel),
                            mm_d[:, 0:gel], gix[:, jlo * 8:(jlo + nt) * 8],
                            nt * 128, nregs[nt], gel,
                            elem_step=HID)
                        gtiles[ci] = gt
                        issued[0] += 1
                    return gtiles[k]

                def lterm(i, pacc2, has_tiles):
                    for q in range(nq):
                        for h in range(fin):
                            nc.tensor.matmul(
                                out=pacc2[q][:],
                                lhsT=lcq[:, (c * 4 + h * 2 + q) * 128:
                                         (c * 4 + h * 2 + q + 1) * 128],
                                rhs=xs(src_tile, h, i),
                                start=(h == 0),
                                stop=(h == fin - 1) and not has_tiles)

                def finish(i, pacc2, started):
                    for q in range(nq):
                        pq = pacc2[q][:]
                        if dst_mode in ('A', 'B'):
                            dbuf = A if dst_mode == 'A' else Bb
                            nc.scalar.activation(
                                out=dbuf[:, q * NP + i * 128:
                                         q * NP + (i + 1) * 128],
                                in_=pq,
                                func=mybir.ActivationFunctionType.Relu,
                                bias=(bcol[:, c * 4 + q: c * 4 + q + 1]
                                      if BIAS_AP else 0.0))
                        elif dst_mode == 'resid':
                            # A = 0.5*A + relu(0.5*pacc + 0.5*b)
                            asl = A[:, q * NP + i * 128: q * NP + (i + 1) * 128]
                            t = rstg.tile([128, 128], bf, tag="rt", name="rt")
                            nc.scalar.activation(
                                out=t[:], in_=pq,
                                func=mybir.ActivationFunctionType.Relu,
                                scale=0.5,
                                bias=bcol[:, c * 4 + 2 + q: c * 4 + 2 + q + 1])
                            nc.vector.tensor_scalar_mul(asl, asl, 0.5)
                            nc.vector.tensor_tensor(
                                out=asl, in0=asl, in1=t[:],
                                op=mybir.AluOpType.add)
                        else:  # final
                            t = rstg.tile([128, 128], bf, tag="tt", name="tt")
                            nc.scalar.activation(
                                out=t[0:3, :], in_=pacc2[0][0:3, :],
                                func=mybir.ActivationFunctionType.Tanh,
                                bias=bcol[0:3, c * 4: c * 4 + 1])
                            fpend.append((i, t))

                def flush_final(i):
                    while fpend and fpend[0][0] <= i:
                        fi, t = fpend.pop(0)
                        pt = ptp.tile([128, 128], bf, tag="pt", name="pt")
                        nc.tensor.transpose(
                            out=pt[:, 0:3], in_=t[0:3, :],
                            identity=idbf[0:3, 0:3])
                        nc.scalar.mul(
                            out=ostage[:, fi * 3:(fi + 1) * 3],
                            in_=pt[:, 0:3], mul=0.1)

                for i in range(NB):
                    pacc2 = [acc.tile([128, 128], f32, tag="pacc", name="pacc")
                             for _ in range(nq)]
                    tj = tiles_of.get(i, [])
                    lterm(i, pacc2, bool(tj))
                    for j in tj:
                        k = int(tile_chunk[j])
                        gt = need_chunk(k)
                        jj = j - chunks[k][0]
                        last = j == tj[-1]
                        for q in range(nq):
                            nc.tensor.matmul(
                                out=pacc2[q][:],
                                lhsT=gt[:, jj * gel + q * 128:
                                        jj * gel + (q + 1) * 128],
                                rhs=S_all[:, j * 128:(j + 1) * 128],
                                start=False, stop=last)
                    finish(i, pacc2, [True, True])
                    if final:
                        flush_final(i - MLAG)
                    if nxt is not None and i >= MLAG:
                        emit_m_block(nxt, i - MLAG)
                if nxt is not None:
                    for i in range(NB - MLAG, NB):
                        emit_m_block(nxt, i)
                if final:
                    flush_final(NB)

                if final:
                    nfull = N // 128  # 53 full blocks
                    nc.gpsimd.dma_start(
                        out=out_d[0:nfull * 128, :].rearrange(
                            "(i p) c -> p i c", p=128),
                        in_=ostage[:, 0:nfull * 3].rearrange(
                            "p (i c) -> p i c", c=3))
                    rem = N - nfull * 128
                    nc.gpsimd.dma_start(
                        out=out_d[nfull * 128:N, :],
                        in_=ostage[0:rem, nfull * 3:(nfull + 1) * 3])

            # ---- network ----
            ncv = 10 if DEBUG_STAGE == 0 else {1: 1, 2: 2, 3: 3, 9: 9}[DEBUG_STAGE]
            states = [None] * (NCONV + 1)
            states[0] = m_state(0, _src_of(0, A, Bb), mm_pp[0])
            for i in range(NB):
                emit_m_block(states[0], i)
            for c in range(ncv):
                # Pool-issued SBUF->SBUF DMA reading the sentinel: Q7 waits
                # for the sentinel HWDGE write (FIFO after all mm writes), so
                # every later gather sees a fully-written mm buffer.
                nc.gpsimd.dma_start(out=sdm[:], in_=snt[:])
                nxt = None
                if c + 1 < ncv:
                    states[c + 1] = m_state(c + 1, _src_of(c + 1, A, Bb),
                                            mm_pp[(c + 1) % 2])
                    if PIPELINE:
                        nxt = states[c + 1]
                conv_gs(c, _src_of(c, A, Bb), _dst_of(c), mm_pp[c % 2], nxt)
                if not PIPELINE and c + 1 < ncv:
                    for i in range(NB):
                        emit_m_block(states[c + 1], i)
            if DEBUG_STAGE:
                dsrc = A if DEBUG_STAGE in (1, 3, 9) else Bb
                nc.sync.dma_start(out=dbg_d[:], in_=dsrc[:])

    nc.finalize()
    return nc


_CACHE = {}
TRACE = False
LAST_RESULTS = None


def _make_dispatch(nc, n_cores):
    """Build a cached PJRT dispatcher (mirrors bass2jax.run_bass_via_pjrt but
    traces/compiles the jitted callable once instead of per call)."""
    import jax
    from jax.sharding import Mesh, PartitionSpec
    from jax.experimental.shard_map import shard_map
    from concourse import bass2jax
    import concourse.mybir as mb

    bass2jax.install_neuronx_cc_hook()
    partition_name = (nc.partition_id_tensor.name
                      if nc.partition_id_tensor else None)
    in_names, out_names, out_avals, zero_outs = [], [], [], []
    for alloc in nc.m.functions[0].allocations:
        if not isinstance(alloc, mb.MemoryLocationSet):
            continue
        name = alloc.memorylocations[0].name
        if alloc.kind == "ExternalInput":
            if name != partition_name:
                in_names.append(name)
        elif alloc.kind == "ExternalOutput":
            shape = tuple(alloc.tensor_shape)
            dtype = mb.dt.np(alloc.dtype)
            out_names.append(name)
            out_avals.append(jax.core.ShapedArray(shape, dtype))
            zero_outs.append(np.zeros(shape, dtype))
    n_params = len(in_names)
    n_outs = len(out_avals)
    all_in_names = list(in_names) + list(out_names)
    if partition_name is not None:
        all_in_names.append(partition_name)
    donate = tuple(range(n_params, n_params + n_outs))

    def _body(*args):
        operands = list(args)
        if partition_name is not None:
            operands.append(bass2jax.partition_id_tensor())
        outs = bass2jax._bass_exec_p.bind(
            *operands,
            out_avals=tuple(out_avals),
            in_names=tuple(all_in_names),
            out_names=tuple(out_names),
            lowering_input_output_aliases=(),
            sim_require_finite=True,
            sim_require_nnan=True,
            nc=nc,
        )
        return tuple(outs)

    devices = jax.devices()[:n_cores]
    mesh = Mesh(np.asarray(devices), ("core",))
    in_specs = (PartitionSpec("core"),) * (n_params + n_outs)
    out_specs = (PartitionSpec("core"),) * n_outs
    sharded = jax.jit(
        shard_map(_body, mesh=mesh, in_specs=in_specs, out_specs=out_specs,
                  check_rep=False),
        donate_argnums=donate, keep_unused=True)

    from jax.sharding import NamedSharding
    shard = NamedSharding(mesh, PartitionSpec("core"))
    dev_cache = {}   # name -> (digest, device array); replicated inputs only

    def run(in_maps):
        import hashlib
        concat_in = []
        for i, name in enumerate(in_names):
            if name == "x0T":
                concat_in.append(np.concatenate(
                    [np.asarray(m[name]) for m in in_maps], axis=0))
                continue
            # identical across cores: keep device-resident, keyed by content
            arr = np.asarray(in_maps[0][name])
            dig = hashlib.md5(arr.tobytes()).hexdigest()
            hit = dev_cache.get(name)
            if hit is None or hit[0] != dig:
                ga = jax.device_put(
                    np.concatenate([arr] * n_cores, axis=0), shard)
                dev_cache[name] = (dig, ga)
            concat_in.append(dev_cache[name][1])
        concat_zeros = [
            np.zeros((n_cores * z.shape[0], *z.shape[1:]), z.dtype)
            for z in zero_outs]
        out_arrs = sharded(*concat_in, *concat_zeros)
        return [
            {name: np.asarray(out_arrs[i]).reshape(
                n_cores, *out_avals[i].shape)[c]
             for i, name in enumerate(out_names)}
            for c in range(n_cores)]

    return run


def _host_arrays(inputs):
    src = np.asarray(inputs["edge_src"]).astype(np.int64)
    dst = np.asarray(inputs["edge_dst"]).astype(np.int64)
    val = np.asarray(inputs["edge_val"], np.float32)

    gidx, S, tile_block = _edge_tiles(src, dst, val)
    KT = len(tile_block)
    s_host = np.ascontiguousarray(
        S.transpose(1, 0, 2).reshape(128, KT * 128)).astype(BF16)
    gidx_w = np.ascontiguousarray(
        np.tile(gidx.reshape(KT * 8, 16).T, (8, 1)))

    wcat = np.zeros((128, NCONV * 2 * HID), np.float32)
    lcq = np.zeros((128, NCONV * 4 * 128), np.float32)
    # bcol layout per conv c: col c*4+q = b[q-half]; col c*4+2+q = 0.5*b
    bcol = np.zeros((128, NCONV * 4), np.float32)

    def put(c, W, L, b):
        nh = W.shape[0] // 128
        no = W.shape[1]
        for h in range(nh):
            wcat[:, (2 * c + h) * HID:(2 * c + h) * HID + no] = \
                W[h * 128:(h + 1) * 128]
            for q in range(2):
                qs = slice(q * 128, min((q + 1) * 128, no))
                ncol = qs.stop - qs.start
                if ncol <= 0:
                    continue
                lcq[:, (c * 4 + h * 2 + q) * 128:
                    (c * 4 + h * 2 + q) * 128 + ncol] = \
                    L[h * 128:(h + 1) * 128, qs]
        for q in range(2):
            qs = slice(q * 128, min((q + 1) * 128, len(b)))
            ncol = qs.stop - qs.start
            if ncol <= 0:
                continue
            bcol[0:ncol, c * 4 + q] = b[qs]
            bcol[0:ncol, c * 4 + 2 + q] = 0.5 * b[qs]

    put(0, np.asarray(inputs["W1"], np.float32),
        np.asarray(inputs["L1"], np.float32),
        np.asarray(inputs["b1"], np.float32))
    Wb = np.asarray(inputs["Wb"], np.float32)
    Lb = np.asarray(inputs["Lb"], np.float32)
    bb = np.asarray(inputs["bb"], np.float32)
    for k in range(8):
        put(1 + k, Wb[k], Lb[k], bb[k])
    put(9, np.asarray(inputs["W2"], np.float32),
        np.asarray(inputs["L2"], np.float32),
        np.asarray(inputs["b2"], np.float32))

    common = {
        "wcat": wcat.astype(BF16), "lcq": lcq.astype(BF16),
        "bcol": bcol, "smat": s_host, "gidx": gidx_w,
    }
    return common, tile_block, KT


def kernel(**inputs):
    verts = np.asarray(inputs["verts_feats"], np.float32)   # [8, 6890, 128]
    Bsz = verts.shape[0]
    common, tile_block, KT = _host_arrays(inputs)

    key = (KT, tuple(tile_block))
    if key not in _CACHE:
        _CACHE.clear()
        nc = _build_program(tile_block, KT)
        _CACHE[key] = (nc, _make_dispatch(nc, Bsz))
    nc, dispatch = _CACHE[key]

    x0T = np.zeros((Bsz, 128, NP), BF16)
    x0T[:, :, :N] = verts.transpose(0, 2, 1)
    in_maps = [dict(common, x0T=x0T[b]) for b in range(Bsz)]
    if TRACE:
        res = bass_utils.run_bass_kernel_spmd(
            nc, in_maps, core_ids=list(range(Bsz)), trace=True)
        globals()['LAST_RESULTS'] = res
        results = res.results
    else:
        results = dispatch(in_maps)
    out = np.stack([results[b]["out"] for b in range(Bsz)], axis=0)
    return out.astype(np.float32)


if __name__ == "__main__":
    sys.path.insert(0, os.path.dirname(os.path.abspath(__file__)))
    import reference as R
    inputs = {k: np.asarray(v) for k, v in R.setup_inputs().items()}
    exp = np.asarray(R.reference(**R.setup_inputs()))
    got = kernel(**inputs)
    err = np.abs(got - exp).max() / np.abs(exp).max()
    print("Relative error:", err)
